# revision 21
# baseline (speedup 1.0000x reference)
"""Distributed Trainium2 kernel for nn_Attention_54795192762650.

GQA attention block with the reference's "scrambled" row-major head
reshapes. 8 NeuronCores: data-parallel over batch (2) x tensor-parallel
over kv-head pairs (4). Because the reference reshapes mix the token and
channel axes, a head's Q slab depends on only 64 token-rows of x but ALL
columns of W_q — so x (token rows) is sharded per core and the weights
are replicated.

Per core (b = cid//4, c = cid%4, kv heads {2c, 2c+1}):
  - QKV projection of the core's token rows (bf16 matmuls, fp32 PSUM)
  - layout shuffles to [d, token] / [j, d] forms (one big block-diagonal
    DVE stream-transpose + block-move copies; one DRAM round-trip for V)
  - scores S^T[j, l] per head pair via tile_position row-split (K_c=64)
  - causal tile skipping; exp on ScalarE; PV matmul against V padded
    with 64 ones-columns so softmax denominators arrive pre-broadcast
  - per l-quarter: AllGather (bf16) of normalized O^T over the 4-core
    batch group, then the output projection of the gathered O^T against
    the core's 512-column shard of (row-permuted) W_out — quarters
    pipeline against the next quarter's attention.

Host side only shards/concats (plus dtype casts and a W_out row
permutation matching the on-device channel stacking order).
"""

import sys

import numpy as np

if "/opt/trn_rl_repo" not in sys.path:
    sys.path.insert(0, "/opt/trn_rl_repo")

import ml_dtypes

B, L, D, HD = 2, 2048, 2048, 64
NKV, NG, NH = 8, 4, 32
P = 128
FD = 512          # matmul moving free dim (one fp32 PSUM bank)
KT = D // P       # 16 contraction tiles
NEG = np.float32(-8e9)  # 8 * (-1e9); exp((s+NEG)/8) == 0 in fp32

_NC_CACHE = {}


def _build(causal: bool):
    import concourse.bacc as bacc
    import concourse.tile as tile
    from concourse import mybir

    f32 = mybir.dt.float32
    b16 = mybir.dt.bfloat16
    Exp = mybir.ActivationFunctionType.Exp
    add = mybir.AluOpType.add
    mult = mybir.AluOpType.mult

    nc = bacc.Bacc("TRN2", target_bir_lowering=False, debug=False, num_devices=8)

    xq = nc.dram_tensor("xq", [D, 512], b16, kind="ExternalInput")
    xkv = nc.dram_tensor("xkv", [D, 512], b16, kind="ExternalInput")
    wq = nc.dram_tensor("wq", [D, D], b16, kind="ExternalInput")
    wk = nc.dram_tensor("wk", [D, 512], b16, kind="ExternalInput")
    wv = nc.dram_tensor("wv", [D, 512], b16, kind="ExternalInput")
    wo = nc.dram_tensor("wo", [D, 512], b16, kind="ExternalInput")
    mtmpl = nc.dram_tensor("mtmpl", [P, 896], b16, kind="ExternalInput")
    eye = nc.dram_tensor("eye", [P, P], b16, kind="ExternalInput")
    if not causal:
        mt8 = nc.dram_tensor("mt8", [L, L], b16, kind="ExternalInput")
    out = nc.dram_tensor("out", [L, 512], f32, kind="ExternalOutput")

    RG = [[0, 1, 2, 3], [4, 5, 6, 7]]

    with tile.TileContext(nc) as tc:
        with tc.tile_pool(name="pres", bufs=1) as pres, \
             tc.tile_pool(name="wpool", bufs=4) as wpool, \
             tc.tile_pool(name="pearly", bufs=1) as pearly, \
             tc.tile_pool(name="dram", bufs=1, space="DRAM") as dram:
            mt_sb = pres.tile([P, 896], b16, name="mt_sb", tag="mt_sb")
            nc.sync.dma_start(mt_sb[:], mtmpl[:])
            eye_sb = pres.tile([P, P], b16, name="eye_sb", tag="eye_sb")
            nc.sync.dma_start(eye_sb[:], eye[:])
            otn = pres.tile([64, 2 * NG, L], b16, name="otn", tag="otn")  # [d, (g,hd), l]
            wo_sb = pres.tile([P, KT, FD], b16, name="wo_sb", tag="wo_sb")
            nc.sync.dma_start(wo_sb[:], wo.rearrange("(ct p) m -> p ct m", p=P))
            if not causal:
                mt8_sb = pres.tile([P, KT, L], b16, name="mt8_sb", tag="mt8_sb")
                nc.sync.dma_start(mt8_sb[:], mt8.rearrange("(jt p) l -> p jt l", p=P))

            slabk = [pearly.tile([P, 2, 512], b16, name=f"slabk{h}", tag=f"slabk{h}")
                     for h in range(2)]
            slabv = [pearly.tile([P, 2, 512], b16, name=f"slabv{h}", tag=f"slabv{h}")
                     for h in range(2)]
            slabq = [pearly.tile([P, L], b16, name=f"slabq{g}", tag=f"slabq{g}")
                     for g in range(NG)]
            kt_sb = pearly.tile([P, L], b16, name="kt_sb", tag="kt_sb")
            v_sb = [pearly.tile([P, KT, P], b16, name=f"v_sb{h}", tag=f"v_sb{h}")
                    for h in range(2)]
            qt_sb = [pearly.tile([P, L], b16, name=f"qt_sb{g}", tag=f"qt_sb{g}")
                     for g in range(NG)]

            # ------------- phase A: projections (xpool closes after) -------------
            with tc.tile_pool(name="xpool", bufs=1) as xpool:
                xq_sb = xpool.tile([P, KT, 512], b16, name="xq_sb", tag="xq_sb")
                nc.sync.dma_start(xq_sb[:], xq.rearrange("(kt p) c -> p kt c", p=P))
                xkv_sb = xpool.tile([P, KT, 512], b16, name="xkv_sb", tag="xkv_sb")
                nc.sync.dma_start(xkv_sb[:], xkv.rearrange("(kt p) c -> p kt c", p=P))

                with tc.tile_pool(name="pskv", bufs=8, space="PSUM") as pskv:
                    pk = {}
                    for hb in range(2):
                        for th in range(2):
                            pk[("k", hb, th)] = pskv.tile([P, FD], f32,
                                                          name=f"pk{hb}{th}", tag="pj")
                            pk[("v", hb, th)] = pskv.tile([P, FD], f32,
                                                          name=f"pv{hb}{th}", tag="pj")
                    for kt in range(KT):
                        wk_t = wpool.tile([P, FD], b16, name="wk_t", tag="wk_t")
                        nc.sync.dma_start(wk_t[:], wk[kt * P:(kt + 1) * P, :])
                        wv_t = wpool.tile([P, FD], b16, name="wv_t", tag="wv_t")
                        nc.sync.dma_start(wv_t[:], wv[kt * P:(kt + 1) * P, :])
                        for hb in range(2):
                            for th in range(2):
                                lhsT = xkv_sb[:, kt, hb * 256 + th * P: hb * 256 + (th + 1) * P]
                                nc.tensor.matmul(pk[("k", hb, th)][:], lhsT, wk_t[:],
                                                 start=(kt == 0), stop=(kt == KT - 1))
                                nc.tensor.matmul(pk[("v", hb, th)][:], lhsT, wv_t[:],
                                                 start=(kt == 0), stop=(kt == KT - 1))
                    for hb in range(2):
                        for th in range(2):
                            nc.scalar.copy(slabk[hb][:, th, :], pk[("k", hb, th)][:])
                            nc.scalar.copy(slabv[hb][:, th, :], pk[("v", hb, th)][:])

                with tc.tile_pool(name="psq", bufs=8, space="PSUM") as psq:
                    for cc in range(4):
                        pq = [psq.tile([P, FD], f32, name=f"pq{g}", tag="pq")
                              for g in range(NG)]
                        for kt in range(KT):
                            wq_t = wpool.tile([P, FD], b16, name="wq_t", tag="wq_t")
                            nc.sync.dma_start(
                                wq_t[:], wq[kt * P:(kt + 1) * P, cc * FD:(cc + 1) * FD])
                            for g in range(NG):
                                lhsT = xq_sb[:, kt, g * P:(g + 1) * P]
                                nc.tensor.matmul(pq[g][:], lhsT, wq_t[:],
                                                 start=(kt == 0), stop=(kt == KT - 1))
                        for g in range(NG):
                            nc.scalar.copy(slabq[g][:, cc * FD:(cc + 1) * FD], pq[g][:])

            # ------------- phase B: layout shuffles -------------
            with tc.tile_pool(name="apool", bufs=1) as apool:
                # K: one block-diagonal transpose per (hb, th), then block moves.
                # KT_sb[64*hb + d, j] = K_hb[j, d],  j = t*8 + u
                for hb in range(2):
                    for th in range(2):
                        kst = apool.tile([P, FD], b16, name="kst", tag="kst", bufs=2)
                        nc.vector.transpose(kst[:], slabk[hb][:, th, :])
                        for tl in range(4):
                            for be in range(2):
                                src = kst[32 * tl:32 * tl + 32, :].rearrange(
                                    "p (u bd) -> p u bd", u=8)[:, :, 32 * be:32 * be + 32]
                                o_base = th * 1024 + tl * 256
                                dst = kt_sb[64 * hb + 32 * be: 64 * hb + 32 * be + 32,
                                            o_base:o_base + 256].rearrange(
                                    "p (tt u) -> p u tt", u=8)
                                nc.vector.tensor_copy(dst, src)
                # V via DRAM round trip; ones-columns 64:128 make the PV matmul
                # emit softmax denominators pre-broadcast on PSUM rows 64:128.
                for hb in range(2):
                    vsc = dram.tile([256, 512], b16, name=f"vsc{hb}", tag=f"vsc{hb}")
                    for th in range(2):
                        nc.sync.dma_start(vsc[th * P:(th + 1) * P, :], slabv[hb][:, th, :])
                    nc.sync.dma_start(
                        v_sb[hb][:, :, 0:64],
                        vsc.rearrange("(jt tl) (u d) -> (tl u) jt d", tl=16, u=8))
                    nc.vector.memset(v_sb[hb][:, :, 64:128], 1.0)
                # Q: per-(pair, c-chunk) block-diagonal transpose + block moves
                # (pipelines against the tail of the Q projection).
                # QT_sb[g][64*hd + d, l] = Q_(pair g, hd)[l, d],  l = t'*32 + u
                for g in range(NG):
                    qst = apool.tile([P, L], b16, name=f"qst{g}", tag=f"qst{g}", bufs=1)
                    for cc in range(4):
                        nc.vector.transpose(qst[:, cc * FD:(cc + 1) * FD],
                                            slabq[g][:, cc * FD:(cc + 1) * FD])
                        for hd in range(2):
                            for tl in range(2):
                                for be in range(2):
                                    src = qst[64 * hd + 32 * tl: 64 * hd + 32 * tl + 32,
                                              cc * FD:(cc + 1) * FD].rearrange(
                                        "p (u bd) -> p u bd", u=8)[:, :, 32 * be:32 * be + 32]
                                    dst = qt_sb[g][64 * hd + 32 * be: 64 * hd + 32 * be + 32,
                                                   tl * 1024:(tl + 1) * 1024].rearrange(
                                        "p (tt u) -> p u tt", u=32)[:, 8 * cc:8 * cc + 8, :]
                                    if be == 0:
                                        nc.vector.tensor_copy(dst, src)
                                    else:
                                        nc.gpsimd.tensor_copy(dst, src)

                # ------------- phase C/D: attention + chunked AG + out-proj -------------
                agin = [dram.tile([512, FD], b16, name=f"agin{m}", tag=f"agin{m}")
                        for m in range(4)]
                agout = [dram.tile([D, FD], b16, name=f"agout{m}", tag=f"agout{m}")
                         for m in range(4)]
                with tc.tile_pool(name="psc", bufs=1, space="PSUM") as psc:
                    for m in range(4):
                        jt_max = 4 * m + 4 if causal else KT
                        for gp in ((0, 1), (2, 3)):
                            po = {}
                            for ci, gx in enumerate(gp):
                                for hd in range(2):
                                    po[(ci, hd)] = psc.tile(
                                        [P, FD], f32, name=f"po{ci}{hd}", tag="po", bufs=4)
                            ps_cur = [None, None]
                            et_cur = [None, None]
                            et_prev = [None, None]
                            for jt in range(jt_max + 1):
                                for ci, gx in enumerate(gp):
                                    if jt < jt_max:
                                        ps = psc.tile([P, 2 * FD], f32, name=f"ps{ci}",
                                                      tag=f"ps{ci}", bufs=1)
                                        ps_cur[ci] = ps
                                        s_ = jt - 4 * m
                                        for hd in range(2):
                                            sl = ps[:, hd * FD:(hd + 1) * FD]
                                            pre = False
                                            if causal and s_ >= 0:
                                                off = 384 - 128 * s_
                                                nc.tensor.matmul(
                                                    sl, eye_sb[:], mt_sb[:, off:off + FD],
                                                    start=True, stop=False)
                                                pre = True
                                            elif not causal:
                                                nc.tensor.matmul(
                                                    sl, eye_sb[:],
                                                    mt8_sb[:, jt, m * FD:(m + 1) * FD],
                                                    start=True, stop=False)
                                                pre = True
                                            nc.tensor.matmul(
                                                sl,
                                                kt_sb[64 * hd:64 * hd + 64, jt * P:(jt + 1) * P],
                                                qt_sb[gx][64 * hd:64 * hd + 64,
                                                          m * FD:(m + 1) * FD],
                                                start=not pre, stop=True,
                                                tile_position=(64 * hd, 0))
                                        e_t = apool.tile([P, 2 * FD], b16, name=f"e{ci}",
                                                         tag="e_t", bufs=4)
                                        nc.scalar.activation(e_t[:], ps[:], Exp, scale=0.125)
                                        et_cur[ci] = e_t
                                    if jt >= 1:
                                        jp = jt - 1
                                        for hd in range(2):
                                            nc.tensor.matmul(
                                                po[(ci, hd)][:, :], v_sb[hd][:, jp, :],
                                                et_prev[ci][:, hd * FD:(hd + 1) * FD],
                                                start=(jp == 0), stop=(jp == jt_max - 1))
                                for ci in range(2):
                                    et_prev[ci] = et_cur[ci]
                            for ci, gx in enumerate(gp):
                                for hd in range(2):
                                    sden = apool.tile([64, FD], f32, name="sden",
                                                      tag="sden", bufs=3)
                                    nc.scalar.copy(sden[:], po[(ci, hd)][64:128, :])
                                    srec = apool.tile([64, FD], f32, name="srec",
                                                      tag="srec", bufs=3)
                                    nc.vector.reciprocal_approx_fast(srec[:], sden[:])
                                    nc.vector.tensor_tensor(
                                        otn[:, gx * 2 + hd, m * FD:(m + 1) * FD],
                                        po[(ci, hd)][0:64, :], srec[:], mult)
                                    nc.sync.dma_start(
                                        agin[m][(gx * 2 + hd) * 64:(gx * 2 + hd + 1) * 64, :],
                                        otn[:, gx * 2 + hd, m * FD:(m + 1) * FD])

                        # all pairs done for this l-quarter: gather O^T across the
                        # batch group and run its output-projection slice.
                        nc.gpsimd.collective_compute(
                            "AllGather", mybir.AluOpType.bypass, replica_groups=RG,
                            ins=[agin[m].opt()], outs=[agout[m].opt()])
                        ot_m = apool.tile([P, KT, FD], b16, name="ot_m",
                                          tag="ot_m", bufs=2)
                        nc.sync.dma_start(
                            ot_m[:], agout[m].rearrange("(ct p) l -> p ct l", p=P))
                        for lt in range(4):
                            py = psc.tile([P, FD], f32, name="py", tag="po", bufs=4)
                            for ct in range(KT):
                                nc.tensor.matmul(
                                    py[:], ot_m[:, ct, lt * P:(lt + 1) * P],
                                    wo_sb[:, ct, :],
                                    start=(ct == 0), stop=(ct == KT - 1))
                            y_sb = apool.tile([P, FD], f32, name="y_sb",
                                              tag="y_sb", bufs=2)
                            nc.scalar.copy(y_sb[:], py[:])
                            nc.sync.dma_start(
                                out[(4 * m + lt) * P:(4 * m + lt + 1) * P, :],
                                y_sb[:])

    nc.compile()
    return nc


def _get_nc(causal: bool):
    if causal not in _NC_CACHE:
        _NC_CACHE[causal] = _build(causal)
    return _NC_CACHE[causal]


def kernel(x, mask, W_qkv, W_out):
    from concourse.bass_utils import run_bass_kernel_spmd

    bf = ml_dtypes.bfloat16
    x = np.asarray(x, dtype=np.float32)
    mask = np.asarray(mask, dtype=np.float32)
    W_qkv = np.asarray(W_qkv, dtype=np.float32)
    W_out = np.asarray(W_out, dtype=np.float32)

    xT = np.ascontiguousarray(x.transpose(0, 2, 1)).astype(bf)  # [B, k, l]
    Wq = np.ascontiguousarray(W_qkv[:, :2048]).astype(bf)
    Wk = np.ascontiguousarray(W_qkv[:, 2048:2560]).astype(bf)
    Wv = np.ascontiguousarray(W_qkv[:, 2560:3072]).astype(bf)

    # W_out rows permuted to the on-device channel stacking order (c, g, hd, d)
    perm = np.empty(D, dtype=np.int64)
    i = 0
    for c in range(4):
        for g in range(NG):
            for hb in range(2):
                base = g * 512 + (2 * c + hb) * 64
                perm[i:i + 64] = np.arange(base, base + 64)
                i += 64
    wo_perm = W_out[perm, :].astype(bf)

    tril = np.tril(np.ones((L, L), dtype=bool))
    expected = np.where(tril, np.float32(0.0), np.float32(-1e9))
    causal = bool(np.array_equal(mask, expected))

    pp = np.arange(P)[:, None]
    qq = np.arange(896)[None, :]
    mtmpl = np.where(pp > qq - 384, NEG, np.float32(0.0)).astype(bf)
    eyem = np.eye(P, dtype=np.float32).astype(bf)

    in_maps = []
    for cid in range(8):
        b, c = divmod(cid, 4)
        h0 = 2 * c
        qrows = np.concatenate(
            [np.arange(64 * (8 * g + h0), 64 * (8 * g + h0) + 128) for g in range(NG)])
        im = {
            "xq": np.ascontiguousarray(xT[b][:, qrows]),
            "xkv": np.ascontiguousarray(xT[b][:, 512 * c:512 * c + 512]),
            "wq": Wq, "wk": Wk, "wv": Wv,
            "wo": np.ascontiguousarray(wo_perm[:, 512 * c:512 * c + 512]),
            "mtmpl": mtmpl, "eye": eyem,
        }
        if not causal:
            im["mt8"] = np.ascontiguousarray(8.0 * mask.T).astype(bf)
        in_maps.append(im)

    nc = _get_nc(causal)
    res = run_bass_kernel_spmd(nc, in_maps, list(range(8)))
    outp = np.empty((B, L, D), dtype=np.float32)
    for cid in range(8):
        b, c = divmod(cid, 4)
        outp[b, :, 512 * c:512 * c + 512] = res.results[cid]["out"]
    return outp


# revision 22
# speedup vs baseline: 1.2325x; 1.2325x over previous
"""Distributed Trainium2 kernel for nn_Attention_54795192762650.

GQA attention block with the reference's "scrambled" row-major head
reshapes. 8 NeuronCores: data-parallel over batch (2) x tensor-parallel
over kv-head pairs (4). Because the reference reshapes mix the token and
channel axes, a head's Q slab depends on only 64 token-rows of x but ALL
columns of W_q — so x (token rows) is sharded per core and the weights
are replicated.

Per core (b = cid//4, c = cid%4, kv heads {2c, 2c+1}):
  - QKV projection of the core's token rows (bf16 matmuls, fp32 PSUM)
  - layout shuffles to [d, token] / [j, d] forms (one big block-diagonal
    DVE stream-transpose + block-move copies; one DRAM round-trip for V)
  - scores S^T[j, l] per head pair via tile_position row-split (K_c=64)
  - causal tile skipping; exp on ScalarE; PV matmul against V padded
    with 64 ones-columns so softmax denominators arrive pre-broadcast
  - per l-quarter: AllGather (bf16) of normalized O^T over the 4-core
    batch group, then the output projection of the gathered O^T against
    the core's 512-column shard of (row-permuted) W_out — quarters
    pipeline against the next quarter's attention.

Host side only shards/concats (plus dtype casts and a W_out row
permutation matching the on-device channel stacking order).
"""

import sys

import numpy as np

if "/opt/trn_rl_repo" not in sys.path:
    sys.path.insert(0, "/opt/trn_rl_repo")

import ml_dtypes

B, L, D, HD = 2, 2048, 2048, 64
NKV, NG, NH = 8, 4, 32
P = 128
FD = 512          # matmul moving free dim (one fp32 PSUM bank)
KT = D // P       # 16 contraction tiles
NEG = np.float32(-8e9)  # 8 * (-1e9); exp((s+NEG)/8) == 0 in fp32

_NC_CACHE = {}


def _build(causal: bool):
    import concourse.bacc as bacc
    import concourse.tile as tile
    from concourse import mybir

    f32 = mybir.dt.float32
    b16 = mybir.dt.bfloat16
    Exp = mybir.ActivationFunctionType.Exp
    add = mybir.AluOpType.add
    mult = mybir.AluOpType.mult

    nc = bacc.Bacc("TRN2", target_bir_lowering=False, debug=False, num_devices=8)

    xq = nc.dram_tensor("xq", [D, 512], b16, kind="ExternalInput")
    xkv = nc.dram_tensor("xkv", [D, 512], b16, kind="ExternalInput")
    wq = nc.dram_tensor("wq", [D, D], b16, kind="ExternalInput")
    wk = nc.dram_tensor("wk", [D, 512], b16, kind="ExternalInput")
    wv = nc.dram_tensor("wv", [D, 512], b16, kind="ExternalInput")
    wo = nc.dram_tensor("wo", [D, 512], b16, kind="ExternalInput")
    mtmpl = nc.dram_tensor("mtmpl", [P, 896], b16, kind="ExternalInput")
    eye = nc.dram_tensor("eye", [P, P], b16, kind="ExternalInput")
    if not causal:
        mt8 = nc.dram_tensor("mt8", [L, L], b16, kind="ExternalInput")
    out = nc.dram_tensor("out", [L, 512], f32, kind="ExternalOutput")

    RG = [[0, 1, 2, 3], [4, 5, 6, 7]]

    with tile.TileContext(nc) as tc:
        with tc.tile_pool(name="pres", bufs=1) as pres, \
             tc.tile_pool(name="wpool", bufs=4) as wpool, \
             tc.tile_pool(name="pearly", bufs=1) as pearly, \
             tc.tile_pool(name="dram", bufs=1, space="DRAM") as dram:
            mt_sb = pres.tile([P, 896], b16, name="mt_sb", tag="mt_sb")
            nc.sync.dma_start(mt_sb[:], mtmpl[:])
            eye_sb = pres.tile([P, P], b16, name="eye_sb", tag="eye_sb")
            nc.sync.dma_start(eye_sb[:], eye[:])
            otn = pres.tile([64, 2 * NG, L], b16, name="otn", tag="otn")  # [d, (g,hd), l]
            wo_sb = pres.tile([P, KT, FD], b16, name="wo_sb", tag="wo_sb")
            nc.sync.dma_start(wo_sb[:], wo.rearrange("(ct p) m -> p ct m", p=P))
            if not causal:
                mt8_sb = pres.tile([P, KT, L], b16, name="mt8_sb", tag="mt8_sb")
                nc.sync.dma_start(mt8_sb[:], mt8.rearrange("(jt p) l -> p jt l", p=P))

            slabk = [pearly.tile([P, 2, 512], b16, name=f"slabk{h}", tag=f"slabk{h}")
                     for h in range(2)]
            slabv = [pearly.tile([P, 2, 512], b16, name=f"slabv{h}", tag=f"slabv{h}")
                     for h in range(2)]
            slabq = [pearly.tile([P, L], b16, name=f"slabq{g}", tag=f"slabq{g}")
                     for g in range(NG)]
            kt_sb = pearly.tile([P, L], b16, name="kt_sb", tag="kt_sb")
            v_sb = [pearly.tile([P, KT, P], b16, name=f"v_sb{h}", tag=f"v_sb{h}")
                    for h in range(2)]
            qt_sb = [pearly.tile([P, L], b16, name=f"qt_sb{g}", tag=f"qt_sb{g}")
                     for g in range(NG)]

            # ------------- phase A: projections (xpool closes after) -------------
            with tc.tile_pool(name="xpool", bufs=1) as xpool:
                xq_sb = xpool.tile([P, KT, 512], b16, name="xq_sb", tag="xq_sb")
                nc.sync.dma_start(xq_sb[:], xq.rearrange("(kt p) c -> p kt c", p=P))
                xkv_sb = xpool.tile([P, KT, 512], b16, name="xkv_sb", tag="xkv_sb")
                nc.sync.dma_start(xkv_sb[:], xkv.rearrange("(kt p) c -> p kt c", p=P))

                with tc.tile_pool(name="pskv", bufs=8, space="PSUM") as pskv:
                    pk = {}
                    for hb in range(2):
                        for th in range(2):
                            pk[("k", hb, th)] = pskv.tile([P, FD], f32,
                                                          name=f"pk{hb}{th}", tag="pj")
                            pk[("v", hb, th)] = pskv.tile([P, FD], f32,
                                                          name=f"pv{hb}{th}", tag="pj")
                    for kt in range(KT):
                        wk_t = wpool.tile([P, FD], b16, name="wk_t", tag="wk_t")
                        nc.sync.dma_start(wk_t[:], wk[kt * P:(kt + 1) * P, :])
                        wv_t = wpool.tile([P, FD], b16, name="wv_t", tag="wv_t")
                        nc.sync.dma_start(wv_t[:], wv[kt * P:(kt + 1) * P, :])
                        for hb in range(2):
                            for th in range(2):
                                lhsT = xkv_sb[:, kt, hb * 256 + th * P: hb * 256 + (th + 1) * P]
                                nc.tensor.matmul(pk[("k", hb, th)][:], lhsT, wk_t[:],
                                                 start=(kt == 0), stop=(kt == KT - 1))
                                nc.tensor.matmul(pk[("v", hb, th)][:], lhsT, wv_t[:],
                                                 start=(kt == 0), stop=(kt == KT - 1))
                    for hb in range(2):
                        for th in range(2):
                            nc.scalar.copy(slabk[hb][:, th, :], pk[("k", hb, th)][:])
                            nc.scalar.copy(slabv[hb][:, th, :], pk[("v", hb, th)][:])

                with tc.tile_pool(name="psq", bufs=8, space="PSUM") as psq:
                    for cc in range(4):
                        pq = [psq.tile([P, FD], f32, name=f"pq{g}", tag="pq")
                              for g in range(NG)]
                        for kt in range(KT):
                            wq_t = wpool.tile([P, FD], b16, name="wq_t", tag="wq_t")
                            nc.sync.dma_start(
                                wq_t[:], wq[kt * P:(kt + 1) * P, cc * FD:(cc + 1) * FD])
                            for g in range(NG):
                                lhsT = xq_sb[:, kt, g * P:(g + 1) * P]
                                nc.tensor.matmul(pq[g][:], lhsT, wq_t[:],
                                                 start=(kt == 0), stop=(kt == KT - 1))
                        for g in range(NG):
                            nc.scalar.copy(slabq[g][:, cc * FD:(cc + 1) * FD], pq[g][:])

            # ------------- phase B: layout shuffles -------------
            with tc.tile_pool(name="apool", bufs=1) as apool:
                # K: one block-diagonal transpose per (hb, th), then block moves.
                # KT_sb[64*hb + d, j] = K_hb[j, d],  j = t*8 + u
                for hb in range(2):
                    for th in range(2):
                        kst = apool.tile([P, FD], b16, name="kst", tag="kst", bufs=2)
                        nc.vector.transpose(kst[:], slabk[hb][:, th, :])
                        for tl in range(4):
                            for be in range(2):
                                src = kst[32 * tl:32 * tl + 32, :].rearrange(
                                    "p (u bd) -> p u bd", u=8)[:, :, 32 * be:32 * be + 32]
                                o_base = th * 1024 + tl * 256
                                dst = kt_sb[64 * hb + 32 * be: 64 * hb + 32 * be + 32,
                                            o_base:o_base + 256].rearrange(
                                    "p (tt u) -> p u tt", u=8)
                                nc.vector.tensor_copy(dst, src)
                # V via DRAM round trip; ones-columns 64:128 make the PV matmul
                # emit softmax denominators pre-broadcast on PSUM rows 64:128.
                for hb in range(2):
                    vsc = dram.tile([256, 512], b16, name=f"vsc{hb}", tag=f"vsc{hb}")
                    for th in range(2):
                        nc.sync.dma_start(vsc[th * P:(th + 1) * P, :], slabv[hb][:, th, :])
                    nc.sync.dma_start(
                        v_sb[hb][:, :, 0:64],
                        vsc.rearrange("(jt tl) (u d) -> (tl u) jt d", tl=16, u=8))
                    nc.vector.memset(v_sb[hb][:, :, 64:128], 1.0)
                # Q: per-(pair, c-chunk) block-diagonal transpose + block moves
                # (pipelines against the tail of the Q projection).
                # QT_sb[g][64*hd + d, l] = Q_(pair g, hd)[l, d],  l = t'*32 + u
                for g in range(NG):
                    qst = apool.tile([P, L], b16, name=f"qst{g}", tag=f"qst{g}", bufs=1)
                    for cc in range(4):
                        nc.vector.transpose(qst[:, cc * FD:(cc + 1) * FD],
                                            slabq[g][:, cc * FD:(cc + 1) * FD])
                        for hd in range(2):
                            for tl in range(2):
                                for be in range(2):
                                    src = qst[64 * hd + 32 * tl: 64 * hd + 32 * tl + 32,
                                              cc * FD:(cc + 1) * FD].rearrange(
                                        "p (u bd) -> p u bd", u=8)[:, :, 32 * be:32 * be + 32]
                                    dst = qt_sb[g][64 * hd + 32 * be: 64 * hd + 32 * be + 32,
                                                   tl * 1024:(tl + 1) * 1024].rearrange(
                                        "p (tt u) -> p u tt", u=32)[:, 8 * cc:8 * cc + 8, :]
                                    if be == 0:
                                        nc.vector.tensor_copy(dst, src)
                                    else:
                                        nc.gpsimd.tensor_copy(dst, src)

                # ------------- phase C/D: attention + chunked AG + out-proj -------------
                agin = [dram.tile([512, FD], b16, name=f"agin{m}", tag=f"agin{m}")
                        for m in range(4)]
                agout = [dram.tile([D, FD], b16, name=f"agout{m}", tag=f"agout{m}")
                         for m in range(4)]
                with tc.tile_pool(name="psc", bufs=1, space="PSUM") as psc:
                    for m in range(4):
                        jt_max = 4 * m + 4 if causal else KT
                        for g in range(NG):
                            po = [psc.tile([P, FD], f32, name=f"po{hd}", tag="po", bufs=3)
                                  for hd in range(2)]
                            hist = {}
                            for jt in range(jt_max + 1):
                                if jt < jt_max:
                                    ps = psc.tile([P, 2 * FD], f32, name="ps",
                                                  tag="ps", bufs=2)
                                    e_t = apool.tile([P, 2 * FD], b16, name="e_t",
                                                     tag="e_t", bufs=4)
                                    s_ = jt - 4 * m
                                    strad = causal and s_ >= 0
                                    z = 128 * s_ if strad else 0  # fully-masked prefix
                                    for hd in range(2):
                                        sl = ps[:, hd * FD + z:(hd + 1) * FD]
                                        pre = False
                                        if strad:
                                            # masked E prefix is never exp'd; zero it
                                            if z:
                                                nc.vector.memset(
                                                    e_t[:, hd * FD:hd * FD + z], 0.0)
                                            nc.tensor.matmul(
                                                sl, eye_sb[:], mt_sb[:, 384:896 - z],
                                                start=True, stop=False)
                                            pre = True
                                        elif not causal:
                                            nc.tensor.matmul(
                                                sl, eye_sb[:],
                                                mt8_sb[:, jt, m * FD + z:(m + 1) * FD],
                                                start=True, stop=False)
                                            pre = True
                                        nc.tensor.matmul(
                                            sl,
                                            kt_sb[64 * hd:64 * hd + 64, jt * P:(jt + 1) * P],
                                            qt_sb[g][64 * hd:64 * hd + 64,
                                                     m * FD + z:(m + 1) * FD],
                                            start=not pre, stop=True,
                                            tile_position=(64 * hd, 0))
                                    if z:
                                        exp_in = ps[:, :].rearrange(
                                            "p (hd l) -> p hd l", hd=2)[:, :, z:]
                                        exp_out = e_t[:, :].rearrange(
                                            "p (hd l) -> p hd l", hd=2)[:, :, z:]
                                        nc.scalar.activation(exp_out, exp_in, Exp,
                                                             scale=0.125)
                                    else:
                                        nc.scalar.activation(e_t[:], ps[:], Exp,
                                                             scale=0.125)
                                    hist[jt] = e_t
                                if jt >= 1:
                                    jp = jt - 1
                                    for hd in range(2):
                                        nc.tensor.matmul(
                                            po[hd][:, :], v_sb[hd][:, jp, :],
                                            hist[jp][:, hd * FD:(hd + 1) * FD],
                                            start=(jp == 0), stop=(jp == jt_max - 1))
                                    del hist[jp]
                            for hd in range(2):
                                sden = apool.tile([64, FD], f32, name="sden",
                                                  tag="sden", bufs=3)
                                nc.scalar.copy(sden[:], po[hd][64:128, :])
                                srec = apool.tile([64, FD], f32, name="srec",
                                                  tag="srec", bufs=3)
                                nc.vector.reciprocal_approx_fast(srec[:], sden[:])
                                nc.vector.tensor_tensor(
                                    otn[:, g * 2 + hd, m * FD:(m + 1) * FD],
                                    po[hd][0:64, :], srec[:], mult)
                                nc.sync.dma_start(
                                    agin[m][(g * 2 + hd) * 64:(g * 2 + hd + 1) * 64, :],
                                    otn[:, g * 2 + hd, m * FD:(m + 1) * FD])

                        # all pairs done for this l-quarter: gather O^T across the
                        # batch group and run its output-projection slice.
                        nc.gpsimd.collective_compute(
                            "AllGather", mybir.AluOpType.bypass, replica_groups=RG,
                            ins=[agin[m].opt()], outs=[agout[m].opt()])
                        ot_m = apool.tile([P, KT, FD], b16, name="ot_m",
                                          tag="ot_m", bufs=2)
                        nc.sync.dma_start(
                            ot_m[:], agout[m].rearrange("(ct p) l -> p ct l", p=P))
                        for lt in range(4):
                            py = psc.tile([P, FD], f32, name="py", tag="py", bufs=1)
                            for ct in range(KT):
                                nc.tensor.matmul(
                                    py[:], ot_m[:, ct, lt * P:(lt + 1) * P],
                                    wo_sb[:, ct, :],
                                    start=(ct == 0), stop=(ct == KT - 1))
                            y_sb = apool.tile([P, FD], f32, name="y_sb",
                                              tag="y_sb", bufs=2)
                            nc.scalar.copy(y_sb[:], py[:])
                            nc.sync.dma_start(
                                out[(4 * m + lt) * P:(4 * m + lt + 1) * P, :],
                                y_sb[:])

    nc.compile()
    return nc


def _get_nc(causal: bool):
    if causal not in _NC_CACHE:
        _NC_CACHE[causal] = _build(causal)
    return _NC_CACHE[causal]


def kernel(x, mask, W_qkv, W_out):
    from concourse.bass_utils import run_bass_kernel_spmd

    bf = ml_dtypes.bfloat16
    x = np.asarray(x, dtype=np.float32)
    mask = np.asarray(mask, dtype=np.float32)
    W_qkv = np.asarray(W_qkv, dtype=np.float32)
    W_out = np.asarray(W_out, dtype=np.float32)

    xT = np.ascontiguousarray(x.transpose(0, 2, 1)).astype(bf)  # [B, k, l]
    Wq = np.ascontiguousarray(W_qkv[:, :2048]).astype(bf)
    Wk = np.ascontiguousarray(W_qkv[:, 2048:2560]).astype(bf)
    Wv = np.ascontiguousarray(W_qkv[:, 2560:3072]).astype(bf)

    # W_out rows permuted to the on-device channel stacking order (c, g, hd, d)
    perm = np.empty(D, dtype=np.int64)
    i = 0
    for c in range(4):
        for g in range(NG):
            for hb in range(2):
                base = g * 512 + (2 * c + hb) * 64
                perm[i:i + 64] = np.arange(base, base + 64)
                i += 64
    wo_perm = W_out[perm, :].astype(bf)

    tril = np.tril(np.ones((L, L), dtype=bool))
    expected = np.where(tril, np.float32(0.0), np.float32(-1e9))
    causal = bool(np.array_equal(mask, expected))

    pp = np.arange(P)[:, None]
    qq = np.arange(896)[None, :]
    mtmpl = np.where(pp > qq - 384, NEG, np.float32(0.0)).astype(bf)
    eyem = np.eye(P, dtype=np.float32).astype(bf)

    in_maps = []
    for cid in range(8):
        b, c = divmod(cid, 4)
        h0 = 2 * c
        qrows = np.concatenate(
            [np.arange(64 * (8 * g + h0), 64 * (8 * g + h0) + 128) for g in range(NG)])
        im = {
            "xq": np.ascontiguousarray(xT[b][:, qrows]),
            "xkv": np.ascontiguousarray(xT[b][:, 512 * c:512 * c + 512]),
            "wq": Wq, "wk": Wk, "wv": Wv,
            "wo": np.ascontiguousarray(wo_perm[:, 512 * c:512 * c + 512]),
            "mtmpl": mtmpl, "eye": eyem,
        }
        if not causal:
            im["mt8"] = np.ascontiguousarray(8.0 * mask.T).astype(bf)
        in_maps.append(im)

    nc = _get_nc(causal)
    res = run_bass_kernel_spmd(nc, in_maps, list(range(8)))
    outp = np.empty((B, L, D), dtype=np.float32)
    for cid in range(8):
        b, c = divmod(cid, 4)
        outp[b, :, 512 * c:512 * c + 512] = res.results[cid]["out"]
    return outp


# revision 24
# speedup vs baseline: 1.2637x; 1.0252x over previous
"""Distributed Trainium2 kernel for nn_Attention_54795192762650.

GQA attention block with the reference's "scrambled" row-major head
reshapes. 8 NeuronCores: data-parallel over batch (2) x tensor-parallel
over kv-head pairs (4). Because the reference reshapes mix the token and
channel axes, a head's Q slab depends on only 64 token-rows of x but ALL
columns of W_q — so x (token rows) is sharded per core and the weights
are replicated.

Per core (b = cid//4, c = cid%4, kv heads {2c, 2c+1}):
  - QKV projection of the core's token rows (bf16 matmuls, fp32 PSUM)
  - layout shuffles to [d, token] / [j, d] forms (one big block-diagonal
    DVE stream-transpose + block-move copies; one DRAM round-trip for V)
  - scores S^T[j, l] per head pair via tile_position row-split (K_c=64)
  - causal tile skipping; exp on ScalarE; PV matmul against V padded
    with 64 ones-columns so softmax denominators arrive pre-broadcast
  - per l-quarter: AllGather (bf16) of normalized O^T over the 4-core
    batch group, then the output projection of the gathered O^T against
    the core's 512-column shard of (row-permuted) W_out — quarters
    pipeline against the next quarter's attention.

Host side only shards/concats (plus dtype casts and a W_out row
permutation matching the on-device channel stacking order).
"""

import sys

import numpy as np

if "/opt/trn_rl_repo" not in sys.path:
    sys.path.insert(0, "/opt/trn_rl_repo")

import ml_dtypes

B, L, D, HD = 2, 2048, 2048, 64
NKV, NG, NH = 8, 4, 32
P = 128
FD = 512          # matmul moving free dim (one fp32 PSUM bank)
KT = D // P       # 16 contraction tiles
NEG = np.float32(-8e9)  # 8 * (-1e9); exp((s+NEG)/8) == 0 in fp32

_NC_CACHE = {}


def _build(causal: bool):
    import concourse.bacc as bacc
    import concourse.tile as tile
    from concourse import mybir

    f32 = mybir.dt.float32
    b16 = mybir.dt.bfloat16
    Exp = mybir.ActivationFunctionType.Exp
    add = mybir.AluOpType.add
    mult = mybir.AluOpType.mult

    nc = bacc.Bacc("TRN2", target_bir_lowering=False, debug=False, num_devices=8)

    xq = nc.dram_tensor("xq", [D, 512], b16, kind="ExternalInput")
    xkv = nc.dram_tensor("xkv", [D, 512], b16, kind="ExternalInput")
    wq = nc.dram_tensor("wq", [D, D], b16, kind="ExternalInput")
    wk = nc.dram_tensor("wk", [D, 512], b16, kind="ExternalInput")
    wv = nc.dram_tensor("wv", [D, 512], b16, kind="ExternalInput")
    wo = nc.dram_tensor("wo", [D, 512], b16, kind="ExternalInput")
    mtmpl = nc.dram_tensor("mtmpl", [P, 896], b16, kind="ExternalInput")
    eye = nc.dram_tensor("eye", [P, P], b16, kind="ExternalInput")
    if not causal:
        mt8 = nc.dram_tensor("mt8", [L, L], b16, kind="ExternalInput")
    out = nc.dram_tensor("out", [L, 512], f32, kind="ExternalOutput")

    RG = [[0, 1, 2, 3], [4, 5, 6, 7]]

    with tile.TileContext(nc) as tc:
        with tc.tile_pool(name="pres", bufs=1) as pres, \
             tc.tile_pool(name="wpool", bufs=4) as wpool, \
             tc.tile_pool(name="pearly", bufs=1) as pearly, \
             tc.tile_pool(name="dram", bufs=1, space="DRAM") as dram:
            mt_sb = pres.tile([P, 896], b16, name="mt_sb", tag="mt_sb")
            nc.sync.dma_start(mt_sb[:], mtmpl[:])
            eye_sb = pres.tile([P, P], b16, name="eye_sb", tag="eye_sb")
            nc.sync.dma_start(eye_sb[:], eye[:])
            otn = pres.tile([64, 2 * NG, L], b16, name="otn", tag="otn")  # [d, (g,hd), l]
            wo_sb = pres.tile([P, KT, FD], b16, name="wo_sb", tag="wo_sb")
            nc.sync.dma_start(wo_sb[:], wo.rearrange("(ct p) m -> p ct m", p=P))
            if not causal:
                mt8_sb = pres.tile([P, KT, L], b16, name="mt8_sb", tag="mt8_sb")
                nc.sync.dma_start(mt8_sb[:], mt8.rearrange("(jt p) l -> p jt l", p=P))

            slabk = [pearly.tile([P, 2, 512], b16, name=f"slabk{h}", tag=f"slabk{h}")
                     for h in range(2)]
            slabv = [pearly.tile([P, 2, 512], b16, name=f"slabv{h}", tag=f"slabv{h}")
                     for h in range(2)]
            slabq = [pearly.tile([P, L], b16, name=f"slabq{g}", tag=f"slabq{g}")
                     for g in range(NG)]
            kt_sb = pearly.tile([P, L], b16, name="kt_sb", tag="kt_sb")
            v_sb = [pearly.tile([P, KT, P], b16, name=f"v_sb{h}", tag=f"v_sb{h}")
                    for h in range(2)]
            qt_sb = [pearly.tile([P, L], b16, name=f"qt_sb{g}", tag=f"qt_sb{g}")
                     for g in range(NG)]

            # ------------- phase A: projections (xpool closes after) -------------
            with tc.tile_pool(name="xpool", bufs=1) as xpool:
                xq_sb = xpool.tile([P, KT, 512], b16, name="xq_sb", tag="xq_sb")
                nc.sync.dma_start(xq_sb[:], xq.rearrange("(kt p) c -> p kt c", p=P))
                xkv_sb = xpool.tile([P, KT, 512], b16, name="xkv_sb", tag="xkv_sb")
                nc.sync.dma_start(xkv_sb[:], xkv.rearrange("(kt p) c -> p kt c", p=P))

                with tc.tile_pool(name="pskv", bufs=8, space="PSUM") as pskv:
                    pk = {}
                    for hb in range(2):
                        for th in range(2):
                            pk[("k", hb, th)] = pskv.tile([P, FD], f32,
                                                          name=f"pk{hb}{th}", tag="pj")
                            pk[("v", hb, th)] = pskv.tile([P, FD], f32,
                                                          name=f"pv{hb}{th}", tag="pj")
                    for kt in range(KT):
                        wk_t = wpool.tile([P, FD], b16, name="wk_t", tag="wk_t")
                        nc.sync.dma_start(wk_t[:], wk[kt * P:(kt + 1) * P, :])
                        wv_t = wpool.tile([P, FD], b16, name="wv_t", tag="wv_t")
                        nc.sync.dma_start(wv_t[:], wv[kt * P:(kt + 1) * P, :])
                        for hb in range(2):
                            for th in range(2):
                                lhsT = xkv_sb[:, kt, hb * 256 + th * P: hb * 256 + (th + 1) * P]
                                nc.tensor.matmul(pk[("k", hb, th)][:], lhsT, wk_t[:],
                                                 start=(kt == 0), stop=(kt == KT - 1))
                                nc.tensor.matmul(pk[("v", hb, th)][:], lhsT, wv_t[:],
                                                 start=(kt == 0), stop=(kt == KT - 1))
                    for hb in range(2):
                        for th in range(2):
                            nc.scalar.copy(slabk[hb][:, th, :], pk[("k", hb, th)][:])
                            nc.scalar.copy(slabv[hb][:, th, :], pk[("v", hb, th)][:])

                with tc.tile_pool(name="psq", bufs=8, space="PSUM") as psq:
                    for cc in range(4):
                        pq = [psq.tile([P, FD], f32, name=f"pq{g}", tag="pq")
                              for g in range(NG)]
                        for kt in range(KT):
                            wq_t = wpool.tile([P, FD], b16, name="wq_t", tag="wq_t")
                            nc.sync.dma_start(
                                wq_t[:], wq[kt * P:(kt + 1) * P, cc * FD:(cc + 1) * FD])
                            for g in range(NG):
                                lhsT = xq_sb[:, kt, g * P:(g + 1) * P]
                                nc.tensor.matmul(pq[g][:], lhsT, wq_t[:],
                                                 start=(kt == 0), stop=(kt == KT - 1))
                        for g in range(NG):
                            nc.scalar.copy(slabq[g][:, cc * FD:(cc + 1) * FD], pq[g][:])

            # ------------- phase B: layout shuffles -------------
            with tc.tile_pool(name="apool", bufs=1) as apool:
                # K: one block-diagonal transpose per (hb, th), then block moves.
                # KT_sb[64*hb + d, j] = K_hb[j, d],  j = t*8 + u
                for hb in range(2):
                    for th in range(2):
                        kst = apool.tile([P, FD], b16, name="kst", tag="kst", bufs=2)
                        nc.vector.transpose(kst[:], slabk[hb][:, th, :])
                        for tl in range(4):
                            for be in range(2):
                                src = kst[32 * tl:32 * tl + 32, :].rearrange(
                                    "p (u bd) -> p u bd", u=8)[:, :, 32 * be:32 * be + 32]
                                o_base = th * 1024 + tl * 256
                                dst = kt_sb[64 * hb + 32 * be: 64 * hb + 32 * be + 32,
                                            o_base:o_base + 256].rearrange(
                                    "p (tt u) -> p u tt", u=8)
                                nc.vector.tensor_copy(dst, src)
                # V via DRAM round trip; ones-columns 64:128 make the PV matmul
                # emit softmax denominators pre-broadcast on PSUM rows 64:128.
                for hb in range(2):
                    vsc = dram.tile([256, 512], b16, name=f"vsc{hb}", tag=f"vsc{hb}")
                    for th in range(2):
                        nc.sync.dma_start(vsc[th * P:(th + 1) * P, :], slabv[hb][:, th, :])
                    nc.sync.dma_start(
                        v_sb[hb][:, :, 0:64],
                        vsc.rearrange("(jt tl) (u d) -> (tl u) jt d", tl=16, u=8))
                    nc.vector.memset(v_sb[hb][:, :, 64:128], 1.0)
                # Q: per-(pair, c-chunk) block-diagonal transpose + block moves
                # (pipelines against the tail of the Q projection).
                # QT_sb[g][64*hd + d, l] = Q_(pair g, hd)[l, d],  l = t'*32 + u
                for g in range(NG):
                    qst = apool.tile([P, L], b16, name=f"qst{g}", tag=f"qst{g}", bufs=1)
                    for cc in range(4):
                        nc.vector.transpose(qst[:, cc * FD:(cc + 1) * FD],
                                            slabq[g][:, cc * FD:(cc + 1) * FD])
                        for hd in range(2):
                            for tl in range(2):
                                for be in range(2):
                                    src = qst[64 * hd + 32 * tl: 64 * hd + 32 * tl + 32,
                                              cc * FD:(cc + 1) * FD].rearrange(
                                        "p (u bd) -> p u bd", u=8)[:, :, 32 * be:32 * be + 32]
                                    dst = qt_sb[g][64 * hd + 32 * be: 64 * hd + 32 * be + 32,
                                                   tl * 1024:(tl + 1) * 1024].rearrange(
                                        "p (tt u) -> p u tt", u=32)[:, 8 * cc:8 * cc + 8, :]
                                    if be == 0:
                                        nc.vector.tensor_copy(dst, src)
                                    else:
                                        nc.gpsimd.tensor_copy(dst, src)

                # ------------- phase C/D: attention + chunked AG + out-proj -------------
                agin = [dram.tile([512, FD], b16, name=f"agin{m}", tag=f"agin{m}")
                        for m in range(4)]
                agout = [dram.tile([D, FD], b16, name=f"agout{m}", tag=f"agout{m}")
                         for m in range(4)]
                with tc.tile_pool(name="psc", bufs=1, space="PSUM") as psc:
                    for m in range(4):
                        jt_max = 4 * m + 4 if causal else KT
                        for g in range(NG):
                            po = [psc.tile([P, FD], f32, name=f"po{hd}", tag="po", bufs=3)
                                  for hd in range(2)]
                            hist = {}
                            for jt in range(jt_max + 1):
                                if jt < jt_max:
                                    ps = psc.tile([P, 2 * FD], f32, name="ps",
                                                  tag="ps", bufs=2)
                                    e_t = apool.tile([P, 2 * FD], b16, name="e_t",
                                                     tag="e_t", bufs=4)
                                    s_ = jt - 4 * m
                                    strad = causal and s_ >= 0
                                    z = 128 * s_ if strad else 0  # fully-masked prefix
                                    for hd in range(2):
                                        sl = ps[:, hd * FD + z:(hd + 1) * FD]
                                        pre = False
                                        if strad:
                                            # masked E prefix is never exp'd; zero it
                                            if z:
                                                nc.vector.memset(
                                                    e_t[:, hd * FD:hd * FD + z], 0.0)
                                            nc.tensor.matmul(
                                                sl, eye_sb[:], mt_sb[:, 384:896 - z],
                                                start=True, stop=False)
                                            pre = True
                                        elif not causal:
                                            nc.tensor.matmul(
                                                sl, eye_sb[:],
                                                mt8_sb[:, jt, m * FD + z:(m + 1) * FD],
                                                start=True, stop=False)
                                            pre = True
                                        nc.tensor.matmul(
                                            sl,
                                            kt_sb[64 * hd:64 * hd + 64, jt * P:(jt + 1) * P],
                                            qt_sb[g][64 * hd:64 * hd + 64,
                                                     m * FD + z:(m + 1) * FD],
                                            start=not pre, stop=True,
                                            tile_position=(64 * hd, 0))
                                    if z:
                                        exp_in = ps[:, :].rearrange(
                                            "p (hd l) -> p hd l", hd=2)[:, :, z:]
                                        exp_out = e_t[:, :].rearrange(
                                            "p (hd l) -> p hd l", hd=2)[:, :, z:]
                                        nc.scalar.activation(exp_out, exp_in, Exp,
                                                             scale=0.125)
                                    else:
                                        nc.scalar.activation(e_t[:], ps[:], Exp,
                                                             scale=0.125)
                                    hist[jt] = e_t
                                if jt >= 1:
                                    jp = jt - 1
                                    for hd in range(2):
                                        nc.tensor.matmul(
                                            po[hd][:, :], v_sb[hd][:, jp, :],
                                            hist[jp][:, hd * FD:(hd + 1) * FD],
                                            start=(jp == 0), stop=(jp == jt_max - 1))
                                    del hist[jp]
                            for hd in range(2):
                                sden = apool.tile([64, FD], f32, name="sden",
                                                  tag="sden", bufs=3)
                                nc.scalar.copy(sden[:], po[hd][64:128, :])
                                srec = apool.tile([64, FD], f32, name="srec",
                                                  tag="srec", bufs=3)
                                nc.vector.reciprocal_approx_fast(srec[:], sden[:])
                                nc.vector.tensor_tensor(
                                    otn[:, g * 2 + hd, m * FD:(m + 1) * FD],
                                    po[hd][0:64, :], srec[:], mult)
                                nc.sync.dma_start(
                                    agin[m][(g * 2 + hd) * 64:(g * 2 + hd + 1) * 64, :],
                                    otn[:, g * 2 + hd, m * FD:(m + 1) * FD])

                        # all pairs done for this l-quarter: gather O^T across the
                        # batch group and run its output-projection slice.
                        nc.gpsimd.collective_compute(
                            "AllGather", mybir.AluOpType.bypass, replica_groups=RG,
                            ins=[agin[m].opt()], outs=[agout[m].opt()])
                        ot_m = apool.tile([P, KT, FD], b16, name="ot_m",
                                          tag="ot_m", bufs=2)
                        nc.sync.dma_start(
                            ot_m[:], agout[m].rearrange("(ct p) l -> p ct l", p=P))
                        for lt in range(4):
                            py = psc.tile([P, FD], f32, name="py", tag="py", bufs=1)
                            for ct in range(KT):
                                nc.tensor.matmul(
                                    py[:], ot_m[:, ct, lt * P:(lt + 1) * P],
                                    wo_sb[:, ct, :],
                                    start=(ct == 0), stop=(ct == KT - 1))
                            y_sb = apool.tile([P, FD], f32, name="y_sb",
                                              tag="y_sb", bufs=2)
                            nc.scalar.copy(y_sb[:], py[:])
                            nc.sync.dma_start(
                                out[(4 * m + lt) * P:(4 * m + lt + 1) * P, :],
                                y_sb[:])

    nc.compile()
    return nc


def _get_nc(causal: bool):
    if causal not in _NC_CACHE:
        _NC_CACHE[causal] = _build(causal)
    return _NC_CACHE[causal]


def kernel(x, mask, W_qkv, W_out):
    from concourse.bass_utils import run_bass_kernel_spmd

    bf = ml_dtypes.bfloat16
    x = np.asarray(x, dtype=np.float32)
    mask = np.asarray(mask, dtype=np.float32)
    W_qkv = np.asarray(W_qkv, dtype=np.float32)
    W_out = np.asarray(W_out, dtype=np.float32)

    xT = np.ascontiguousarray(x.transpose(0, 2, 1)).astype(bf)  # [B, k, l]
    Wq = np.ascontiguousarray(W_qkv[:, :2048]).astype(bf)
    Wk = np.ascontiguousarray(W_qkv[:, 2048:2560]).astype(bf)
    Wv = np.ascontiguousarray(W_qkv[:, 2560:3072]).astype(bf)

    # W_out rows permuted to the on-device channel stacking order (c, g, hd, d)
    perm = np.empty(D, dtype=np.int64)
    i = 0
    for c in range(4):
        for g in range(NG):
            for hb in range(2):
                base = g * 512 + (2 * c + hb) * 64
                perm[i:i + 64] = np.arange(base, base + 64)
                i += 64
    wo_perm = W_out[perm, :].astype(bf)

    tril = np.tril(np.ones((L, L), dtype=bool))
    expected = np.where(tril, np.float32(0.0), np.float32(-1e9))
    causal = bool(np.array_equal(mask, expected))

    pp = np.arange(P)[:, None]
    qq = np.arange(896)[None, :]
    mtmpl = np.where(pp > qq - 384, NEG, np.float32(0.0)).astype(bf)
    eyem = np.eye(P, dtype=np.float32).astype(bf)

    in_maps = []
    for cid in range(8):
        b, c = divmod(cid, 4)
        h0 = 2 * c
        qrows = np.concatenate(
            [np.arange(64 * (8 * g + h0), 64 * (8 * g + h0) + 128) for g in range(NG)])
        im = {
            "xq": np.ascontiguousarray(xT[b][:, qrows]),
            "xkv": np.ascontiguousarray(xT[b][:, 512 * c:512 * c + 512]),
            "wq": Wq, "wk": Wk, "wv": Wv,
            "wo": np.ascontiguousarray(wo_perm[:, 512 * c:512 * c + 512]),
            "mtmpl": mtmpl, "eye": eyem,
        }
        if not causal:
            im["mt8"] = np.ascontiguousarray(8.0 * mask.T).astype(bf)
        in_maps.append(im)

    nc = _get_nc(causal)
    res = run_bass_kernel_spmd(nc, in_maps, list(range(8)))
    outp = np.empty((B, L, D), dtype=np.float32)
    for cid in range(8):
        b, c = divmod(cid, 4)
        outp[b, :, 512 * c:512 * c + 512] = res.results[cid]["out"]
    return outp
